# revision 6
# baseline (speedup 1.0000x reference)
"""Ernie4.5 attention block (T=2048, H=4096, 32 Q heads / 8 KV heads, rope,
causal, o_proj) on 8 Trainium2 NeuronCores.

Sharding: tensor-parallel by head. Each core computes QKV for its 4 Q heads +
1 KV head (column-sharded w_qkv), runs attention for those heads, AllGathers
the (transposed, bf16) attention outputs per 512-column t-chunk, and computes
a 512-column shard of o_proj (column-sharded w_o). Host only does layout
work: transpose/permute/shard inputs, concatenate output shards.

v3 schedule:
- all matmul operands bf16 (FWL halves LDWEIGHTS, halves input DMA); fp32
  PSUM accumulation; rel err ~3e-3.
- QKV processed in chunk PAIRS: one LDWEIGHTS feeds two N=512 matmuls.
  PSUM tag `big` is shared between QKV accumulators (4 live) and attention
  score tiles (2 live) since the phases alternate.
- rope fused straight out of PSUM (no DRAM round-trip).
- attention chunk c follows QKV pair once its k/v columns exist.
- diagonal attention blocks are causally trimmed: QK/exp/PV only touch
  columns >= 128*kd of the chunk.
- softmax denominator: DVE-accumulated exp sum, one ones-matmul per
  (head, chunk).
- o_proj at the end over all 4 chunks; AllGathers hide under it.
"""

import numpy as np

import concourse.bacc as bacc
import concourse.mybir as mybir
import concourse.tile as tile
from concourse.bass_utils import run_bass_kernel_spmd
from concourse.masks import make_identity

T = 2048
HID = 4096
NH = 32
NKV = 8
HD = 128
THETA = 500000.0
SCALE = HD ** -0.5
N_CORES = 8
HPC = NH // N_CORES          # q heads per core
KS = HID // 128              # 32 k-subtiles
CH = 512                     # t-chunk width (QKV, attention, o_proj)
NCH = T // CH                # 4 chunks
SB = CH // 128               # 128-blocks per chunk (4)

F32 = mybir.dt.float32
F32R = mybir.dt.float32r
BF16 = mybir.dt.bfloat16

QKV_M = HPC + 2              # m-tiles per chunk: 4 q heads, k, v


def _emit_body(nc, tc, io, rep):
    hid8, wq6, c1_d, c2_d, masks_d, wo_d, outT_d = io

    with (
        tc.tile_pool(name=f"p0{rep}", bufs=1) as p0,
        tc.tile_pool(name=f"dram{rep}", bufs=1, space="DRAM") as dpool,
    ):
        # ---- persistent tiles ----
        masks = p0.tile([128, SB, CH], BF16)
        ident = p0.tile([128, 128], BF16)
        ones_f = p0.tile([128, 1], F32)
        ones = p0.tile([128, 1], F32R)
        qT = [p0.tile([128, T], BF16, tag=f"qT{h}", name=f"qT{h}")
              for h in range(HPC)]
        kT = p0.tile([128, T], BF16, tag="kT")
        v_nat = p0.tile([128, T // 128, 128], BF16, tag="vnat")
        ag_outs = []

        with (
            tc.tile_pool(name=f"p1{rep}", bufs=1) as p1,
            tc.tile_pool(name=f"psab{rep}", bufs=1, space="PSUM") as psab,
        ):
            # weights + first two hidden chunks first on the DMA queue
            wt = []
            for m in range(QKV_M):
                w = p1.tile([128, KS, 128], BF16, tag=f"wt{m}")
                nc.sync.dma_start(w[:], wq6[m])
                wt.append(w)

            def load_hid(ci):
                ht = p1.tile([128, KS, CH], BF16, tag="hid", bufs=3)
                nc.sync.dma_start(ht[:], hid8[ci])
                return ht

            hts = {0: load_hid(0), 1: load_hid(1)}

            # constants (needed only once rope/attention starts)
            nc.sync.dma_start(masks[:],
                              masks_d.rearrange("(k p) t -> p k t", p=128))
            c1 = p1.tile([128, T], BF16, tag="c1")
            c2 = p1.tile([128, T], BF16, tag="c2")
            nc.sync.dma_start(c1[:], c1_d[:])
            nc.sync.dma_start(c2[:], c2_d[:])
            make_identity(nc, ident[:])
            nc.vector.memset(ones_f[:], 1.0)
            nc.vector.tensor_copy(ones[:], ones_f[:])

            def rope(ps, dst, c):
                # dst[:, c*CH:(c+1)*CH] (bf16) <- rope(ps) with tables c1/c2
                cs = slice(c * CH, (c + 1) * CH)
                sw = p1.tile([128, CH], F32, tag="rswap", bufs=2)
                nc.vector.tensor_copy(sw[0:64], ps[64:128])
                nc.vector.tensor_copy(sw[64:128], ps[0:64])
                ta = p1.tile([128, CH], F32, tag="ropetmp", bufs=1)
                nc.vector.tensor_tensor(ta[:], ps[:], c1[:, cs],
                                        mybir.AluOpType.mult)
                nc.vector.tensor_tensor(sw[:], sw[:], c2[:, cs],
                                        mybir.AluOpType.mult)
                nc.vector.tensor_tensor(dst[:, cs], ta[:], sw[:],
                                        mybir.AluOpType.add)

            def qkv_pair(c0):
                # chunks c0, c0+1 share each LDWEIGHTS
                for m in range(QKV_M):
                    pss = []
                    for c in (c0, c0 + 1):
                        ps = psab.tile([128, CH], F32, tag="big", bufs=4,
                                       name=f"ps{c}")
                        pss.append(ps)
                    for k in range(KS):
                        for ci, c in enumerate((c0, c0 + 1)):
                            nc.tensor.matmul(
                                pss[ci][:], wt[m][:, k], hts[c][:, k],
                                start=(k == 0), stop=(k == KS - 1))
                    for ci, c in enumerate((c0, c0 + 1)):
                        ps = pss[ci]
                        if m < HPC:
                            rope(ps, qT[m], c)
                        elif m == HPC:
                            rope(ps, kT, c)
                        else:
                            vraw = p1.tile([128, CH], BF16, tag="vraw",
                                           bufs=2)
                            nc.vector.tensor_copy(vraw[:], ps[:])
                            for jj in range(SB):
                                tp = psab.tile([128, 128], BF16, tag="tp",
                                               bufs=1)
                                nc.tensor.transpose(
                                    tp[:], vraw[:, jj * 128:(jj + 1) * 128],
                                    ident[:])
                                nc.vector.tensor_copy(v_nat[:, c * SB + jj],
                                                      tp[:])

            def attention(c):
                nj = SB * (c + 1)
                ag_in = dpool.tile([HPC * 128, CH], BF16, tag="agin", bufs=2)
                for h in range(HPC):
                    pv = psab.tile([128, CH], F32, tag="pv", bufs=2)
                    esum = p1.tile([128, CH], F32R, tag="esum", bufs=2)
                    for j in range(nj):
                        kd = j - SB * c
                        w0 = 128 * kd if kd > 0 else 0
                        wsl = slice(w0, CH)
                        qsl = slice(c * CH + w0, (c + 1) * CH)
                        sc = psab.tile([128, CH], F32, tag="big", bufs=4,
                                       name="sc")
                        nc.tensor.matmul(
                            sc[:, wsl], kT[:, j * 128:(j + 1) * 128],
                            qT[h][:, qsl], start=True, stop=True)
                        e = p1.tile([128, CH], BF16, tag="expT", bufs=3)
                        if kd >= 0:
                            # diagonal block: exp then causal mask-mult
                            e0 = p1.tile([128, CH], F32, tag="expTmp",
                                         bufs=1)
                            nc.scalar.activation(
                                e0[:, wsl], sc[:, wsl],
                                mybir.ActivationFunctionType.Exp, scale=SCALE)
                            nc.vector.tensor_tensor(
                                e[:, wsl], e0[:, wsl], masks[:, kd, wsl],
                                mybir.AluOpType.mult)
                        else:
                            nc.scalar.activation(
                                e[:, wsl], sc[:, wsl],
                                mybir.ActivationFunctionType.Exp, scale=SCALE)
                        nc.tensor.matmul(pv[:, wsl], v_nat[:, j], e[:, wsl],
                                         start=(j == 0), stop=(j == nj - 1))
                        if j == 0:
                            nc.vector.tensor_copy(esum[:], e[:])
                        else:
                            nc.vector.tensor_tensor(
                                esum[:, wsl], esum[:, wsl], e[:, wsl],
                                mybir.AluOpType.add)
                    zp = psab.tile([1, CH], F32, tag="zp", bufs=1)
                    nc.tensor.matmul(zp[:], ones[:], esum[:],
                                     start=True, stop=True)
                    zr = p1.tile([1, CH], F32, tag="zr", bufs=2)
                    nc.vector.reciprocal(zr[:], zp[:])
                    zb = p1.tile([128, CH], F32, tag="zb", bufs=1)
                    nc.gpsimd.partition_broadcast(zb[:], zr[:])
                    at = p1.tile([128, CH], BF16, tag="attnT", bufs=2)
                    nc.vector.tensor_tensor(at[:], pv[:], zb[:],
                                            mybir.AluOpType.mult)
                    nc.sync.dma_start(ag_in[h * 128:(h + 1) * 128, :], at[:])

                ag_out = dpool.tile([NH * HD, CH], BF16, tag=f"agout{c}",
                                    bufs=1, addr_space="Shared")
                nc.gpsimd.collective_compute(
                    "AllGather",
                    mybir.AluOpType.bypass,
                    replica_groups=[list(range(N_CORES))],
                    ins=[ag_in[:].opt()],
                    outs=[ag_out[:].opt()],
                )
                ag_outs.append(ag_out)

            qkv_pair(0)
            hts[2] = load_hid(2)
            hts[3] = load_hid(3)
            attention(0)
            attention(1)
            qkv_pair(2)
            attention(2)
            attention(3)

        # ---- o_proj over all chunks ----
        with (
            tc.tile_pool(name=f"p2{rep}", bufs=1) as p2,
            tc.tile_pool(name=f"psc{rep}", bufs=1, space="PSUM") as psc,
        ):
            wot = p2.tile([128, KS, 512], BF16)
            nc.sync.dma_start(wot[:], wo_d[:])
            for c in range(NCH):
                ag_re = ag_outs[c].rearrange("(ko ki) t -> ki ko t", ki=128)
                rt = p2.tile([128, KS, CH], BF16, tag="agsb", bufs=2)
                nc.sync.dma_start(rt[:], ag_re[:])
                for m in range(4):
                    po = psc.tile([128, CH], F32, tag="po", bufs=2)
                    for k in range(KS):
                        nc.tensor.matmul(
                            po[:], wot[:, k, m * 128:(m + 1) * 128], rt[:, k],
                            start=(k == 0), stop=(k == KS - 1))
                    oo = p2.tile([128, CH], F32, tag="oout", bufs=2)
                    nc.vector.tensor_copy(oo[:], po[:])
                    nc.sync.dma_start(
                        outT_d[m * 128:(m + 1) * 128, c * CH:(c + 1) * CH],
                        oo[:])


def build_program(reps=1):
    nc = bacc.Bacc("TRN2", target_bir_lowering=False, debug=False,
                   num_devices=N_CORES)
    hid8 = nc.dram_tensor("hid8", [NCH, 128, KS, CH], BF16,
                          kind="ExternalInput")
    wq6 = nc.dram_tensor("wq6", [QKV_M, 128, KS, 128], BF16,
                         kind="ExternalInput")
    c1_d = nc.dram_tensor("c1", [128, T], BF16, kind="ExternalInput")
    c2_d = nc.dram_tensor("c2", [128, T], BF16, kind="ExternalInput")
    masks_d = nc.dram_tensor("masks", [SB * 128, CH], BF16,
                             kind="ExternalInput")
    wo_d = nc.dram_tensor("wo", [128, KS, 512], BF16, kind="ExternalInput")
    outT_d = nc.dram_tensor("outT", [512, T], F32, kind="ExternalOutput")
    io = (hid8, wq6, c1_d, c2_d, masks_d, wo_d, outT_d)
    with tile.TileContext(nc) as tc:
        for rep in range(reps):
            _emit_body(nc, tc, io, rep)
    nc.compile()
    return nc


def make_core_inputs(positions, hidden_states, w_qkv, w_o):
    """Host-side layout prep. Returns list of per-core input dicts."""
    positions = np.asarray(positions)
    hidden_states = np.asarray(hidden_states, dtype=np.float32)
    w_qkv = np.asarray(w_qkv, dtype=np.float32)
    w_o = np.asarray(w_o, dtype=np.float32)
    assert np.all(np.diff(positions.astype(np.int64)) > 0), (
        "kernel assumes strictly increasing positions (causal mask == index mask)"
    )
    bf16 = mybir.dt.np(BF16)

    # rope tables
    half = HD // 2
    inv_freq = 1.0 / (THETA ** (np.arange(0, half, dtype=np.float32) * 2.0 / HD))
    ang = positions.astype(np.float32)[:, None] * inv_freq[None, :]  # [T, 64]
    cosT = np.cos(ang).T.astype(np.float32)  # [64, T]
    sinT = np.sin(ang).T.astype(np.float32)
    c1 = np.ascontiguousarray(np.concatenate([cosT, cosT], axis=0)).astype(bf16)
    c2 = np.ascontiguousarray(np.concatenate([-sinT, sinT], axis=0)).astype(bf16)

    # diagonal-block causal masks
    masks = np.zeros((SB, 128, CH), dtype=np.float32)
    s_idx = np.arange(128)[:, None]
    t_idx = np.arange(CH)[None, :]
    for k in range(SB):
        masks[k] = (128 * k + s_idx <= t_idx).astype(np.float32)
    masks = masks.reshape(SB * 128, CH).astype(bf16)

    # hidden^T in [ki, ko, t] tiling, pre-chunked
    hidT = hidden_states.T  # [HID, T]
    hid_re = hidT.reshape(KS, 128, T).transpose(1, 0, 2)  # [ki, ko, t]
    hid8 = np.ascontiguousarray(
        hid_re.reshape(128, KS, NCH, CH).transpose(2, 0, 1, 3)
    ).astype(bf16)  # [NCH, 128, KS, CH]

    perm = np.concatenate([np.arange(0, HD, 2), np.arange(1, HD, 2)])

    ins = []
    for r in range(N_CORES):
        cols = []
        for h in range(HPC):
            base = (HPC * r + h) * HD
            cols.append(base + perm)
        cols.append(NH * HD + r * HD + perm)                # k head, permuted
        cols.append(NH * HD + NKV * HD + r * HD + np.arange(HD))  # v head
        cols = np.concatenate(cols)
        wp = w_qkv[:, cols]  # [HID, 768]
        wp_re = wp.reshape(KS, 128, QKV_M * 128).transpose(1, 0, 2)  # ki ko c
        wq6 = np.ascontiguousarray(
            wp_re.reshape(128, KS, QKV_M, 128).transpose(2, 0, 1, 3)
        ).astype(bf16)

        wo_sh = w_o[:, 512 * r:512 * (r + 1)]  # [HID, 512]
        wo_re = np.ascontiguousarray(
            wo_sh.reshape(KS, 128, 512).transpose(1, 0, 2)
        ).astype(bf16)

        ins.append({
            "hid8": hid8, "wq6": wq6, "c1": c1, "c2": c2,
            "masks": masks, "wo": wo_re,
        })
    return ins


_PROGRAM = None


def kernel(positions, hidden_states, w_qkv, w_o):
    global _PROGRAM
    if _PROGRAM is None:
        _PROGRAM = build_program()
    nc = _PROGRAM
    ins = make_core_inputs(positions, hidden_states, w_qkv, w_o)
    res = run_bass_kernel_spmd(nc, ins, list(range(N_CORES)))
    out = np.empty((T, HID), dtype=np.float32)
    for r in range(N_CORES):
        out[:, 512 * r:512 * (r + 1)] = res.results[r]["outT"].T
    return out


# revision 10
# speedup vs baseline: 3.4785x; 3.4785x over previous
"""Ernie4.5 attention block (T=2048, H=4096, 32 Q heads / 8 KV heads, rope,
causal, o_proj) on 8 Trainium2 NeuronCores.

Sharding: tensor-parallel by head. Each core computes QKV for its 4 Q heads +
1 KV head (column-sharded w_qkv), runs attention for those heads, AllGathers
the (transposed, bf16) attention outputs per 512-column t-chunk, and computes
a 512-column shard of o_proj (column-sharded w_o). Host only does layout
work: transpose/permute/shard inputs, concatenate output shards.

v5 schedule:
- all matmul operands bf16 (FWL, halved input DMA); N=512 moving chunks
  (TRN2 matmul output must be fp32; one PSUM bank caps N at 512).
- rope fused straight out of PSUM (no DRAM round-trip).
- attention chunk c follows QKV chunk c; k/v columns exist by causality.
  The next chunk's hidden DMA overlaps compute.
- diagonal attention blocks causally trimmed (QK/exp/PV touch only columns
  >= 128*kd of the chunk) - mostly an exp (Scalar engine) saving.
- softmax denominator: DVE-accumulated exp-sum, one ones-matmul per
  (head, chunk).
- o_proj at the end over all 4 chunks; AllGathers hide under it.
"""

import numpy as np

import concourse.bacc as bacc
import concourse.mybir as mybir
import concourse.tile as tile
from concourse.bass_utils import run_bass_kernel_spmd
from concourse.masks import make_identity

T = 2048
HID = 4096
NH = 32
NKV = 8
HD = 128
THETA = 500000.0
SCALE = HD ** -0.5
N_CORES = 8
HPC = NH // N_CORES          # q heads per core
KS = HID // 128              # 32 k-subtiles
CH = 512                     # attention t-chunk width
NCH = T // CH                # 4 chunks
SB = CH // 128               # 128-blocks per chunk (4)
PW = CH                      # QKV / o_proj moving width (fp32 PSUM cap)
NP = T // PW                 # 4 chunks

F32 = mybir.dt.float32
F32R = mybir.dt.float32r
BF16 = mybir.dt.bfloat16

QKV_M = HPC + 2              # m-tiles per chunk: 4 q heads, k, v


def _emit_body(nc, tc, io, rep):
    hid2, wq6, c1_d, c2_d, masks_d, wo_d, outT_d = io

    with (
        tc.tile_pool(name=f"p0{rep}", bufs=1) as p0,
        tc.tile_pool(name=f"dram{rep}", bufs=1, space="DRAM") as dpool,
    ):
        # ---- persistent tiles ----
        masks = p0.tile([128, SB, CH], BF16)
        ident = p0.tile([128, 128], BF16)
        ones_f = p0.tile([128, 1], F32)
        ones = p0.tile([128, 1], F32R)
        qT = [p0.tile([128, T], BF16, tag=f"qT{h}", name=f"qT{h}")
              for h in range(HPC)]
        kT = p0.tile([128, T], BF16, tag="kT")
        v_nat = p0.tile([128, T // 128, 128], BF16, tag="vnat")
        ag_outs = []

        with (
            tc.tile_pool(name=f"p1{rep}", bufs=1) as p1,
            tc.tile_pool(name=f"psab{rep}", bufs=1, space="PSUM") as psab,
        ):
            # weights + first hidden pair first on the DMA queue
            wt = []
            for m in range(QKV_M):
                w = p1.tile([128, KS, 128], BF16, tag=f"wt{m}")
                nc.sync.dma_start(w[:], wq6[m])
                wt.append(w)

            def load_hid(p):
                ht = p1.tile([128, KS, CH], BF16, tag="hid", bufs=2)
                nc.sync.dma_start(ht[:], hid2[p])
                return ht

            ht = load_hid(0)

            # constants (needed only once rope/attention starts)
            nc.sync.dma_start(masks[:],
                              masks_d.rearrange("(k p) t -> p k t", p=128))
            c1 = p1.tile([128, T], BF16, tag="c1")
            c2 = p1.tile([128, T], BF16, tag="c2")
            nc.sync.dma_start(c1[:], c1_d[:])
            nc.sync.dma_start(c2[:], c2_d[:])
            make_identity(nc, ident[:])
            nc.vector.memset(ones_f[:], 1.0)
            nc.vector.tensor_copy(ones[:], ones_f[:])

            def rope(ps, dst, c):
                # dst[:, c*CH:(c+1)*CH] (bf16) <- rope(ps half) via c1/c2
                cs = slice(c * CH, (c + 1) * CH)
                sw = p1.tile([128, CH], F32, tag="rswap", bufs=2)
                nc.vector.tensor_copy(sw[0:64], ps[64:128])
                nc.vector.tensor_copy(sw[64:128], ps[0:64])
                ta = p1.tile([128, CH], F32, tag="ropetmp", bufs=2)
                nc.vector.tensor_tensor(ta[:], ps[:], c1[:, cs],
                                        mybir.AluOpType.mult)
                nc.vector.tensor_tensor(sw[:], sw[:], c2[:, cs],
                                        mybir.AluOpType.mult)
                nc.vector.tensor_tensor(dst[:, cs], ta[:], sw[:],
                                        mybir.AluOpType.add)

            def qkv_chunk(c, ht):
                for m in range(QKV_M):
                    pq = psab.tile([128, CH], F32, tag="pq", bufs=2)
                    for k in range(KS):
                        nc.tensor.matmul(pq[:], wt[m][:, k], ht[:, k],
                                         start=(k == 0), stop=(k == KS - 1))
                    if m < HPC:
                        rope(pq[:], qT[m], c)
                    elif m == HPC:
                        rope(pq[:], kT, c)
                    else:
                        vraw = p1.tile([128, CH], BF16, tag="vraw", bufs=2)
                        nc.vector.tensor_copy(vraw[:], pq[:])
                        for jj in range(SB):
                            tp = psab.tile([128, 128], BF16, tag="tp",
                                           bufs=1)
                            nc.tensor.transpose(
                                tp[:], vraw[:, jj * 128:(jj + 1) * 128],
                                ident[:])
                            nc.vector.tensor_copy(v_nat[:, c * SB + jj],
                                                  tp[:])

            def attention(c):
                nj = SB * (c + 1)
                ag_in = dpool.tile([HPC * 128, CH], BF16, tag="agin", bufs=2)
                for h in range(HPC):
                    pv = psab.tile([128, CH], F32, tag="pv", bufs=2)
                    esum = p1.tile([128, CH], F32R, tag="esum", bufs=2)
                    for j in range(nj):
                        kd = j - SB * c
                        w0 = 128 * kd if kd > 0 else 0
                        wsl = slice(w0, CH)
                        qsl = slice(c * CH + w0, (c + 1) * CH)
                        sc = psab.tile([128, CH], F32, tag="sc", bufs=2)
                        nc.tensor.matmul(
                            sc[:, wsl], kT[:, j * 128:(j + 1) * 128],
                            qT[h][:, qsl], start=True, stop=True)
                        e = p1.tile([128, CH], BF16, tag="expT", bufs=3)
                        if kd >= 0:
                            # diagonal block: exp then causal mask-mult
                            e0 = p1.tile([128, CH], F32, tag="expTmp",
                                         bufs=2)
                            nc.scalar.activation(
                                e0[:, wsl], sc[:, wsl],
                                mybir.ActivationFunctionType.Exp, scale=SCALE)
                            nc.vector.tensor_tensor(
                                e[:, wsl], e0[:, wsl], masks[:, kd, wsl],
                                mybir.AluOpType.mult)
                        else:
                            nc.scalar.activation(
                                e[:, wsl], sc[:, wsl],
                                mybir.ActivationFunctionType.Exp, scale=SCALE)
                        nc.tensor.matmul(pv[:, wsl], v_nat[:, j], e[:, wsl],
                                         start=(j == 0), stop=(j == nj - 1))
                        if j == 0:
                            nc.vector.tensor_copy(esum[:], e[:])
                        else:
                            nc.vector.tensor_tensor(
                                esum[:, wsl], esum[:, wsl], e[:, wsl],
                                mybir.AluOpType.add)
                    zp = psab.tile([1, CH], F32, tag="zp", bufs=1)
                    nc.tensor.matmul(zp[:], ones[:], esum[:],
                                     start=True, stop=True)
                    zr = p1.tile([1, CH], F32, tag="zr", bufs=2)
                    nc.vector.reciprocal(zr[:], zp[:])
                    zb = p1.tile([128, CH], F32, tag="zb", bufs=2)
                    nc.gpsimd.partition_broadcast(zb[:], zr[:])
                    at = p1.tile([128, CH], BF16, tag="attnT", bufs=2)
                    nc.vector.tensor_tensor(at[:], pv[:], zb[:],
                                            mybir.AluOpType.mult)
                    nc.sync.dma_start(ag_in[h * 128:(h + 1) * 128, :], at[:])

                ag_out = dpool.tile([NH * HD, CH], BF16, tag=f"agout{c}",
                                    bufs=1, addr_space="Shared")
                nc.gpsimd.collective_compute(
                    "AllGather",
                    mybir.AluOpType.bypass,
                    replica_groups=[list(range(N_CORES))],
                    ins=[ag_in[:].opt()],
                    outs=[ag_out[:].opt()],
                )
                ag_outs.append(ag_out)

            for c in range(NCH):
                cur = ht
                if c + 1 < NCH:
                    ht = load_hid(c + 1)
                qkv_chunk(c, cur)
                attention(c)

        # ---- o_proj over chunk pairs ----
        with (
            tc.tile_pool(name=f"p2{rep}", bufs=1) as p2,
            tc.tile_pool(name=f"psc{rep}", bufs=1, space="PSUM") as psc,
        ):
            wot = p2.tile([128, KS, 512], BF16)
            nc.sync.dma_start(wot[:], wo_d[:])
            for c in range(NCH):
                rt = p2.tile([128, KS, CH], BF16, tag="agsb", bufs=2)
                ag_re = ag_outs[c].rearrange("(ko ki) t -> ki ko t", ki=128)
                nc.sync.dma_start(rt[:], ag_re[:])
                for m in range(4):
                    po = psc.tile([128, CH], F32, tag="po", bufs=2)
                    for k in range(KS):
                        nc.tensor.matmul(
                            po[:], wot[:, k, m * 128:(m + 1) * 128], rt[:, k],
                            start=(k == 0), stop=(k == KS - 1))
                    oo = p2.tile([128, CH], F32, tag="oout", bufs=2)
                    nc.vector.tensor_copy(oo[:], po[:])
                    nc.sync.dma_start(
                        outT_d[m * 128:(m + 1) * 128, c * CH:(c + 1) * CH],
                        oo[:])


def build_program(reps=1):
    nc = bacc.Bacc("TRN2", target_bir_lowering=False, debug=False,
                   num_devices=N_CORES)
    hid2 = nc.dram_tensor("hid2", [NCH, 128, KS, CH], BF16,
                          kind="ExternalInput")
    wq6 = nc.dram_tensor("wq6", [QKV_M, 128, KS, 128], BF16,
                         kind="ExternalInput")
    c1_d = nc.dram_tensor("c1", [128, T], BF16, kind="ExternalInput")
    c2_d = nc.dram_tensor("c2", [128, T], BF16, kind="ExternalInput")
    masks_d = nc.dram_tensor("masks", [SB * 128, CH], BF16,
                             kind="ExternalInput")
    wo_d = nc.dram_tensor("wo", [128, KS, 512], BF16, kind="ExternalInput")
    outT_d = nc.dram_tensor("outT", [512, T], F32, kind="ExternalOutput")
    io = (hid2, wq6, c1_d, c2_d, masks_d, wo_d, outT_d)
    with tile.TileContext(nc) as tc:
        for rep in range(reps):
            _emit_body(nc, tc, io, rep)
    nc.compile()
    return nc


def make_core_inputs(positions, hidden_states, w_qkv, w_o):
    """Host-side layout prep. Returns list of per-core input dicts."""
    positions = np.asarray(positions)
    hidden_states = np.asarray(hidden_states, dtype=np.float32)
    w_qkv = np.asarray(w_qkv, dtype=np.float32)
    w_o = np.asarray(w_o, dtype=np.float32)
    assert np.all(np.diff(positions.astype(np.int64)) > 0), (
        "kernel assumes strictly increasing positions (causal mask == index mask)"
    )
    bf16 = mybir.dt.np(BF16)

    # rope tables
    half = HD // 2
    inv_freq = 1.0 / (THETA ** (np.arange(0, half, dtype=np.float32) * 2.0 / HD))
    ang = positions.astype(np.float32)[:, None] * inv_freq[None, :]  # [T, 64]
    cosT = np.cos(ang).T.astype(np.float32)  # [64, T]
    sinT = np.sin(ang).T.astype(np.float32)
    c1 = np.ascontiguousarray(np.concatenate([cosT, cosT], axis=0)).astype(bf16)
    c2 = np.ascontiguousarray(np.concatenate([-sinT, sinT], axis=0)).astype(bf16)

    # diagonal-block causal masks
    masks = np.zeros((SB, 128, CH), dtype=np.float32)
    s_idx = np.arange(128)[:, None]
    t_idx = np.arange(CH)[None, :]
    for k in range(SB):
        masks[k] = (128 * k + s_idx <= t_idx).astype(np.float32)
    masks = masks.reshape(SB * 128, CH).astype(bf16)

    # hidden^T in [ki, ko, t] tiling, pre-chunked in 1024-col pairs
    hidT = hidden_states.T  # [HID, T]
    hid_re = hidT.reshape(KS, 128, T).transpose(1, 0, 2)  # [ki, ko, t]
    hid2 = np.ascontiguousarray(
        hid_re.reshape(128, KS, NCH, CH).transpose(2, 0, 1, 3)
    ).astype(bf16)  # [NCH, 128, KS, CH]

    perm = np.concatenate([np.arange(0, HD, 2), np.arange(1, HD, 2)])

    ins = []
    for r in range(N_CORES):
        cols = []
        for h in range(HPC):
            base = (HPC * r + h) * HD
            cols.append(base + perm)
        cols.append(NH * HD + r * HD + perm)                # k head, permuted
        cols.append(NH * HD + NKV * HD + r * HD + np.arange(HD))  # v head
        cols = np.concatenate(cols)
        wp = w_qkv[:, cols]  # [HID, 768]
        wp_re = wp.reshape(KS, 128, QKV_M * 128).transpose(1, 0, 2)  # ki ko c
        wq6 = np.ascontiguousarray(
            wp_re.reshape(128, KS, QKV_M, 128).transpose(2, 0, 1, 3)
        ).astype(bf16)

        wo_sh = w_o[:, 512 * r:512 * (r + 1)]  # [HID, 512]
        wo_re = np.ascontiguousarray(
            wo_sh.reshape(KS, 128, 512).transpose(1, 0, 2)
        ).astype(bf16)

        ins.append({
            "hid2": hid2, "wq6": wq6, "c1": c1, "c2": c2,
            "masks": masks, "wo": wo_re,
        })
    return ins


_PROGRAM = None


def kernel(positions, hidden_states, w_qkv, w_o):
    global _PROGRAM
    if _PROGRAM is None:
        _PROGRAM = build_program()
    nc = _PROGRAM
    ins = make_core_inputs(positions, hidden_states, w_qkv, w_o)
    res = run_bass_kernel_spmd(nc, ins, list(range(N_CORES)))
    out = np.empty((T, HID), dtype=np.float32)
    for r in range(N_CORES):
        out[:, 512 * r:512 * (r + 1)] = res.results[r]["outT"].T
    return out
